# revision 29
# baseline (speedup 1.0000x reference)
"""DCGRU cell (nn_DCGRUCell) Trainium2 Bass kernel, 8 NeuronCores.

fp8 DoubleRow version. Node dim N=4096 split 8 ways (512 rows/core).
Supports are host-transposed, scaled by 2^11, cast to fp8e4, resident in
SBUF. All diffusion matmuls (hop1/hop2, both GCNs, x-pass) run fp8 with
perf_mode=DoubleRow (2 m-tiles of 128 contraction per instruction).

State (and later the candidate) is resident in SBUF node-major fp8
[q][p][t][c] with c = b*64+h batch-major. hop1 produces node-major own
rows -> fp8 extract -> AllGather (one per support, overlapped with the
next support's sweep) -> hop2 consumes the gathered y1 as DoubleRow
lhsT, producing feature-major y2 directly. Feature-major y1 for the
dense stage comes from PE transposes of the fp8 extracts (64-col blocks
so everything stays at partition base 0). The dense stage runs per-batch
with 5 block matmuls ([74]-row direct+x-diffusion block + 4 [64]-row
y-state blocks) straight out of SBUF-resident staging tiles - no DRAM
round trip. Scale corrections (2^-11 hop1 / 2^-22 hop2) and the
Chebyshev fold (x2 = 2*A@x1 - x0) are folded into W host-side.

kernel(**inputs) takes FULL inputs from reference.setup_inputs() and
returns the FULL [16, 4096, 64] float32 output.
"""
import os
import numpy as np

import concourse.bass as bass
import concourse.mybir as mybir
import concourse.tile as tile
from concourse import bacc
from concourse.bass_utils import run_bass_kernel_spmd

F32 = mybir.dt.float32
BF16 = mybir.dt.bfloat16
F8 = mybir.dt.float8e4
AF = mybir.ActivationFunctionType
DR = mybir.MatmulPerfMode.DoubleRow

NCORES = 8
B, N, H, DIN = 16, 4096, 64, 2
C = DIN + H                 # 66
SN = B * H                  # 1024 state columns
XC = B * DIN                # 32 x columns
NOWN = N // NCORES          # 512
NT = NOWN // 128            # 4
NQ = 8                      # contraction chunks of 4 m-tiles
KD = C + 4 * DIN            # 74 rows of the dense direct block
SA = 2.0 ** 11              # support scale for fp8
GROUP = [list(range(NCORES))]

_NC_CACHE = {}


def build_nc():
    nc = bacc.Bacc("TRN2", target_bir_lowering=False, debug=False,
                   num_devices=NCORES)
    d = {}
    d["Ts"] = nc.dram_tensor("Ts", [2, NQ, 128, NT, NOWN], F8,
                             kind="ExternalInput")
    d["st"] = nc.dram_tensor("st", [NQ, 128, NT, SN], F8,
                             kind="ExternalInput")
    d["xm"] = nc.dram_tensor("xm", [NQ, 128, NT, XC], F8,
                             kind="ExternalInput")
    d["xsT"] = nc.dram_tensor("xsT", [C, B, NOWN], BF16,
                              kind="ExternalInput")
    d["Wg74"] = nc.dram_tensor("Wg74", [KD, 2 * H], BF16,
                               kind="ExternalInput")
    d["Wg64"] = nc.dram_tensor("Wg64", [2 * H, 4, 2 * H], BF16,
                               kind="ExternalInput")
    d["Wu74"] = nc.dram_tensor("Wu74", [KD, H], BF16,
                               kind="ExternalInput")
    d["Wu64"] = nc.dram_tensor("Wu64", [2 * H, 4, H], BF16,
                               kind="ExternalInput")
    d["bg"] = nc.dram_tensor("bg", [2 * H, 1], F32, kind="ExternalInput")
    d["bu"] = nc.dram_tensor("bu", [H, 1], F32, kind="ExternalInput")
    d["outT"] = nc.dram_tensor("outT", [B, H, NOWN], F32,
                               kind="ExternalOutput")

    with tile.TileContext(nc) as tc:
        _emit(nc, tc, d)
    nc.compile()
    return nc


def _emit(nc, tc, d):
    import contextlib
    stack = contextlib.ExitStack()
    with stack:
        const = stack.enter_context(tc.tile_pool(name="const", bufs=1))
        mov = stack.enter_context(tc.tile_pool(name="mov", bufs=1))
        stg = stack.enter_context(tc.tile_pool(name="stg", bufs=1))
        dram = stack.enter_context(
            tc.tile_pool(name="dram", bufs=1, space="DRAM"))
        psum = stack.enter_context(
            tc.tile_pool(name="psum", bufs=1, space="PSUM"))

        # ---------------- identities ----------------
        identf = const.tile([128, 128], F32)
        nc.gpsimd.memset(identf[:], 0.0)
        nc.gpsimd.affine_select(
            out=identf[:], in_=identf[:],
            compare_op=mybir.AluOpType.not_equal, fill=1.0, base=0,
            pattern=[[-1, 128]], channel_multiplier=1)
        identb = const.tile([128, 128], BF16)
        nc.vector.tensor_copy(identb[:], identf[:])
        identf8 = const.tile([128, 128], F8)
        nc.vector.tensor_copy(identf8[:], identf[:])

        # ---------------- resident support tiles ----------------
        Tch = {}
        for s in range(2):
            for q in range(NQ):
                Tch[(s, q)] = const.tile([128, NT, NOWN], F8,
                                         name=f"T{s}_{q}")

        def load_T(s, q):
            nc.scalar.dma_start(Tch[(s, q)][:], d["Ts"].ap()[s, q])

        # ---------------- moving operand tiles ----------------
        # "mov" ring: 8 slots of [128, NT, SN] fp8. GCN1 keeps the state
        # fully resident (loaded once, reused by both support sweeps);
        # GCN2's candidate half-tiles rotate through the same ring.
        def st_loader(sweep_id):
            tiles = {}

            def load(q, eng=None):
                t = mov.tile([128, NT, SN], F8, name=f"mv{sweep_id}_{q}",
                             tag="mov", bufs=NQ)
                (eng or nc.scalar).dma_start(t[:], d["st"].ap()[q])
                tiles[q] = t
            return tiles, load

        mx = {}

        def load_mx(q):
            mx[q] = mov.tile([128, NT, XC], F8, name=f"mx{q}",
                             tag="mx", bufs=NQ)
            nc.scalar.dma_start(mx[q][:], d["xm"].ap()[q])

        st0_tiles, st0_load = st_loader(0)
        st0_load(0, nc.sync)
        load_T(0, 0)
        st0_load(1, nc.sync)
        load_T(0, 1)

        # early sync-ring loads: dense inputs + weights
        xsT74 = stg.tile([KD, B, NOWN], BF16, name="xsT74", tag="xsT",
                         bufs=1)
        nc.sync.dma_start(xsT74[0:C, :, :], d["xsT"].ap())
        wg74 = const.tile([KD, 2 * H], BF16)
        nc.sync.dma_start(wg74[:], d["Wg74"].ap())
        wg64 = const.tile([2 * H, 4, 2 * H], BF16)
        nc.sync.dma_start(wg64[:], d["Wg64"].ap())
        wu74 = const.tile([KD, H], BF16)
        nc.sync.dma_start(wu74[:], d["Wu74"].ap())
        wu64 = const.tile([2 * H, 4, H], BF16)
        nc.sync.dma_start(wu64[:], d["Wu64"].ap())
        bg_t = const.tile([2 * H, 1], F32)
        nc.sync.dma_start(bg_t[:], d["bg"].ap())
        bu_t = const.tile([H, 1], F32)
        nc.sync.dma_start(bu_t[:], d["bu"].ap())

        # ---------------- DRAM staging ----------------
        ag1 = [[dram.tile([128 * NT * SN], F8, name=f"ag1_{g}{s}")
                for s in range(2)] for g in range(2)]
        ag1o = [[dram.tile([NCORES * 128 * NT * SN], F8,
                           name=f"ag1o_{g}{s}", addr_space="Shared")
                 for s in range(2)] for g in range(2)]
        agX = dram.tile([128 * NT * 2 * XC], F8, name="agX")
        agXo = dram.tile([NCORES * 128 * NT * 2 * XC], F8, name="agXo",
                         addr_space="Shared")
        candC = [dram.tile([128 * NT * 512], F8, name=f"candC{i}")
                 for i in range(2)]
        candCo = [dram.tile([NCORES * 128 * NT * 512], F8,
                            name=f"candCo{i}", addr_space="Shared")
                  for i in range(2)]

        def allgather(src, dst):
            nc.gpsimd.collective_compute(
                "AllGather", mybir.AluOpType.bypass, replica_groups=GROUP,
                ins=[src.opt()], outs=[dst.opt()])

        # ---------------- hop-1 sweeps ----------------
        def sweep_psum(g, s):
            ps = {}
            for n in range(NT):
                for ch in range(2):
                    ps[(n, ch)] = psum.tile(
                        [128, 512], F32, name=f"ps{g}{s}{n}{ch}",
                        tag="acc", bufs=8)
            return ps

        def sweep_full(g, s, tiles, load, stagger=None):
            """rhs = full [128, NT, SN] tiles; ch inner (2 MMs/weight)."""
            ps = sweep_psum(g, s)
            for q in range(NQ):
                if load is not None and q < NQ - 2:
                    load(q + 2)
                if stagger is not None:
                    stagger(q)
                for tt in (0, 2):
                    for n in range(NT):
                        lhsT = Tch[(s, q)][:, tt:tt + 2,
                                           n * 128:(n + 1) * 128]
                        for ch in range(2):
                            nc.tensor.matmul(
                                ps[(n, ch)][:], lhsT,
                                tiles[q][:, tt:tt + 2,
                                         ch * 512:(ch + 1) * 512],
                                start=(q == 0 and tt == 0),
                                stop=(q == NQ - 1 and tt == 2),
                                perf_mode=DR)
            return ps

        def load_mc(g2s, ch, q):
            t = mov.tile([128, NT, 512], F8, name=f"mc{g2s}_{ch}{q}",
                         tag="mov", bufs=NQ)
            src = candCo[ch].opt().rearrange(
                "(q p t c) -> q p t c", q=NCORES, t=NT, c=512)
            nc.scalar.dma_start(t[:], src[q])
            return t

        def sweep_g2(s, col_outer):
            """GCN2 sweep: rhs from the two candidate half-gathers."""
            ps = sweep_psum(1, s)
            if col_outer:
                for ch in range(2):
                    tiles = {}
                    tiles[0] = load_mc(s, ch, 0)
                    tiles[1] = load_mc(s, ch, 1)
                    for q in range(NQ):
                        if q < NQ - 2:
                            tiles[q + 2] = load_mc(s, ch, q + 2)
                        for tt in (0, 2):
                            for n in range(NT):
                                nc.tensor.matmul(
                                    ps[(n, ch)][:],
                                    Tch[(1, q)][:, tt:tt + 2,
                                                n * 128:(n + 1) * 128],
                                    tiles[q][:, tt:tt + 2, :],
                                    start=(q == 0 and tt == 0),
                                    stop=(q == NQ - 1 and tt == 2),
                                    perf_mode=DR)
            else:
                tiles = {}
                for ch in range(2):
                    tiles[(ch, 0)] = load_mc(s, ch, 0)
                for q in range(NQ):
                    if q < NQ - 1:
                        for ch in range(2):
                            tiles[(ch, q + 1)] = load_mc(s, ch, q + 1)
                    for tt in (0, 2):
                        for n in range(NT):
                            lhsT = Tch[(1, q)][:, tt:tt + 2,
                                               n * 128:(n + 1) * 128]
                            for ch in range(2):
                                nc.tensor.matmul(
                                    ps[(n, ch)][:], lhsT,
                                    tiles[(ch, q)][:, tt:tt + 2, :],
                                    start=(q == 0 and tt == 0),
                                    stop=(q == NQ - 1 and tt == 2),
                                    perf_mode=DR)
            return ps

        def extract_hop1(g, s, ps):
            exa = stg.tile([128, NT, SN], F8, name=f"exa{g}{s}",
                           tag="exa", bufs=2)
            for n in range(NT):
                for ch in range(2):
                    nc.vector.tensor_copy(
                        exa[:, n, ch * 512:(ch + 1) * 512],
                        ps[(n, ch)][:])
            nc.sync.dma_start(
                ag1[g][s].opt().rearrange("(p t c) -> p t c", t=NT, c=SN),
                exa[:])
            allgather(ag1[g][s], ag1o[g][s])
            return exa

        # ---------------- feature-major y1 via PE transposes ----------
        # y tiles are (b_lo, h)-packed: [128, 8, NOWN] where partition
        # p = (b % 2) * 64 + h and dim1 jb = b // 2. The dense stage
        # reads batch b at partition base (b % 2) * 64 with the W64
        # blocks stored doubled (rows 0:64 and 64:128 identical).
        def y1_stage(g, s, exa):
            yt = stg.tile([128, NQ, NOWN], BF16, name=f"y1t{g}{s}",
                          tag=f"y1t{s}", bufs=1)
            for n in range(NT):
                for jb in range(NQ):
                    # fp8 transpose writes 2-byte PSUM lanes: out element
                    # step must be 2 (walrus checkMatmultOutputs)
                    tp = psum.tile([128, 128, 2], F8, name=f"tp{g}{s}",
                                   tag="acc", bufs=8)
                    nc.tensor.transpose(
                        tp[:, :, 0], exa[:, n, jb * 128:(jb + 1) * 128],
                        identf8[:])
                    nc.vector.tensor_copy(
                        yt[:, jb, n * 128:(n + 1) * 128], tp[:, :, 0])
            return yt

        # ---------------- hop-2 (feature-major out) ----------------
        def hop2_loads(g, s, mrt, qs):
            src = ag1o[g][s].opt().rearrange(
                "(q p t c) -> q p t c", q=NCORES, t=NT, c=SN)
            for q in qs:
                mr = mov.tile([128, NT, SN], F8, name=f"mr{g}{s}{q}",
                              tag="mr", bufs=3)
                nc.scalar.dma_start(mr[:], src[q])
                mrt[q] = mr

        def hop2(g, s):
            mrt = {}
            hop2_loads(g, s, mrt, [0, 1])
            ps = [psum.tile([128, NOWN], F32, name=f"h2_{g}{s}{j}",
                            tag="acc", bufs=8) for j in range(8)]
            for q in range(NQ):
                if q < NQ - 2:
                    hop2_loads(g, s, mrt, [q + 2])
                for tt in (0, 2):
                    for j in range(8):
                        nc.tensor.matmul(
                            ps[j][:],
                            mrt[q][:, tt:tt + 2, j * 128:(j + 1) * 128],
                            Tch[(s, q)][:, tt:tt + 2, :],
                            start=(q == 0 and tt == 0),
                            stop=(q == NQ - 1 and tt == 2),
                            perf_mode=DR)
            yt = stg.tile([128, NQ, NOWN], BF16, name=f"y2t{g}{s}",
                          tag=f"y2t{s}", bufs=1)
            for j in range(8):
                nc.vector.tensor_copy(yt[:, j, :], ps[j][:])
            return yt

        # ---------------- x-column diffusion ----------------
        def x1mm():
            psx = {sx: psum.tile([XC, NOWN], F32, name=f"psx{sx}",
                                 tag="acc", bufs=8) for sx in range(2)}
            for q in range(NQ):
                for tt in (0, 2):
                    for sx in range(2):
                        nc.tensor.matmul(
                            psx[sx][:], mx[q][:, tt:tt + 2, :],
                            Tch[(sx, q)][:, tt:tt + 2, :],
                            start=(q == 0 and tt == 0),
                            stop=(q == NQ - 1 and tt == 2),
                            perf_mode=DR)
            return psx

        def x1post(psx):
            xnm = stg.tile([128, NT, 2, XC], F8, name="xnm", tag="xnm",
                           bufs=1)
            for sx in range(2):
                xe = stg.tile([XC, NOWN], BF16, name=f"xe{sx}", tag="xe",
                              bufs=1)
                nc.vector.tensor_copy(xe[:], psx[sx][:])
                for b in range(B):
                    nc.sync.dma_start(
                        xsT74[C + 4 * sx:C + 4 * sx + DIN, b, :],
                        xe[DIN * b:DIN * (b + 1), :])
                for n in range(NT):
                    tp = psum.tile([128, XC], BF16, name="xtp",
                                   tag="acc", bufs=8)
                    nc.tensor.transpose(
                        tp[:], xe[:, n * 128:(n + 1) * 128],
                        identb[0:XC, 0:XC])
                    nc.vector.tensor_copy(xnm[:, n, sx, :], tp[:])
            nc.sync.dma_start(
                agX.opt().rearrange("(p t s c) -> p t s c", t=NT, s=2,
                                    c=XC),
                xnm[:])
            allgather(agX, agXo)

        def x2pass():
            mrx = stg.tile([128, NQ, NT, 2, XC], F8, name="mrx",
                           tag="mrx", bufs=1)
            nc.scalar.dma_start(
                mrx[:],
                agXo.opt().rearrange("(q p t s c) -> p q t s c",
                                     q=NCORES, t=NT, s=2, c=XC))
            psx2 = {sx: psum.tile([XC, NOWN], F32, name=f"psx2{sx}",
                                  tag="acc", bufs=8) for sx in range(2)}
            for q in range(NQ):
                for tt in (0, 2):
                    for sx in range(2):
                        nc.tensor.matmul(
                            psx2[sx][:], mrx[:, q, tt:tt + 2, sx, :],
                            Tch[(sx, q)][:, tt:tt + 2, :],
                            start=(q == 0 and tt == 0),
                            stop=(q == NQ - 1 and tt == 2),
                            perf_mode=DR)
            for sx in range(2):
                xe2 = stg.tile([XC, NOWN], BF16, name=f"xe2{sx}",
                               tag="xe", bufs=1)
                nc.vector.tensor_copy(xe2[:], psx2[sx][:])
                for b in range(B):
                    nc.sync.dma_start(
                        xsT74[C + 4 * sx + DIN:C + 4 * sx + 2 * DIN,
                              b, :],
                        xe2[DIN * b:DIN * (b + 1), :])

        # ---------------- dense stages ----------------
        def dense_blocks(w74, w64, direct, bloc, y1t, y2t, pst, b0):
            ysrc = [y1t[0], y2t[0], y1t[1], y2t[1]]
            for b2 in range(4):
                nc.tensor.matmul(pst[b2][:], w74[:],
                                 direct[0:KD, bloc + b2, :],
                                 start=True, stop=False)
            for j in range(4):
                for b2 in range(4):
                    b = b0 + b2
                    lo = (b % 2) * H
                    nc.tensor.matmul(
                        pst[b2][:], w64[lo:lo + H, j, :],
                        ysrc[j][lo:lo + H, b // 2, :],
                        start=False, stop=(j == 3))

        def gate_mm(pi, y1t, y2t):
            b0 = 4 * pi
            zps = [psum.tile([2 * H, NOWN], F32, name=f"zps{pi}_{b2}",
                             tag="acc", bufs=8) for b2 in range(4)]
            dense_blocks(wg74, wg64, xsT74, b0, y1t, y2t, zps, b0)
            return zps

        def gate_post(pi, zps, ctn):
            b0 = 4 * pi
            zr = stg.tile([2 * H, 4, NOWN], BF16, name=f"zr{pi}",
                          tag="zr", bufs=2)
            for b2 in range(4):
                nc.scalar.activation(zr[:, b2, :], zps[b2][:],
                                     AF.Sigmoid, bias=bg_t[:])
            rsb = stg.tile([H, 4, NOWN], BF16, name=f"rsb{pi}",
                           tag="rsb", bufs=4)
            nc.scalar.dma_start(rsb[:], zr[H:2 * H, :, :])
            ct74 = stg.tile([KD, 4, NOWN], BF16, name=f"ct74_{pi}",
                            tag="ct74", bufs=4)
            nc.vector.tensor_mul(ct74[0:H, :, :], zr[0:H, :, :],
                                 xsT74[0:H, b0:b0 + 4, :])
            nc.vector.tensor_copy(ct74[H:KD, :, :],
                                  xsT74[H:KD, b0:b0 + 4, :])
            # node-major candidate: half-gather hf = pi//2 accumulates
            # batch groups (pi%2) in a [128, NT, 512] tile, stored + AG'd
            # after the odd pi
            if ctn is None:
                ctn = stg.tile([128, NT, 512], F8, name=f"ctn{pi}",
                               tag="ctn", bufs=1)
            co = (pi % 2) * 256
            for n in range(NT):
                for b2 in range(4):
                    tp = psum.tile([128, H], BF16, name="ctp",
                                   tag="acc", bufs=8)
                    nc.tensor.transpose(
                        tp[:], ct74[0:H, b2, n * 128:(n + 1) * 128],
                        identb[0:H, 0:H])
                    nc.vector.tensor_copy(
                        ctn[:, n, co + b2 * H:co + (b2 + 1) * H], tp[:])
            if pi % 2 == 1:
                hf = pi // 2
                nc.sync.dma_start(
                    candC[hf].opt().rearrange("(p t c) -> p t c", t=NT,
                                              c=512),
                    ctn[:])
                allgather(candC[hf], candCo[hf])
                ctn = None
            return rsb, ct74, ctn

        def update_mm(pi, ct74, y1t, y2t):
            b0 = 4 * pi
            hps = [psum.tile([H, NOWN], F32, name=f"hps{pi}_{b2}",
                             tag="acc", bufs=8) for b2 in range(4)]
            dense_blocks(wu74, wu64, ct74, 0, y1t, y2t, hps, b0)
            return hps

        def update_post(pi, rsb, hps):
            b0 = 4 * pi
            for b2 in range(4):
                hc = stg.tile([H, NOWN], BF16, name=f"hc{pi}{b2}",
                              tag="hc", bufs=2)
                tm = stg.tile([H, NOWN], BF16, name=f"tm{pi}{b2}",
                              tag="tm", bufs=2)
                ot = stg.tile([H, NOWN], F32, name=f"ot{pi}{b2}",
                              tag="ot", bufs=2)
                nc.scalar.activation(hc[:], hps[b2][:], AF.Tanh,
                                     bias=bu_t[:])
                nc.vector.tensor_sub(tm[:], xsT74[0:H, b0 + b2, :],
                                     hc[:])
                nc.vector.tensor_mul(tm[:], rsb[:, b2, :], tm[:])
                nc.vector.tensor_add(ot[:], hc[:], tm[:])
                nc.scalar.dma_start(d["outT"].ap()[b0 + b2], ot[:])

        # ======================= GCN 1 (gate) =======================
        def stagger1(q):
            load_T(1, q)
            if q < NQ - 2:
                load_T(0, q + 2)
            load_mx(q)

        ps = sweep_full(0, 0, st0_tiles, st0_load, stagger=stagger1)
        exa00 = extract_hop1(0, 0, ps)
        psx = x1mm()
        ps = sweep_full(0, 1, st0_tiles, None)
        x1post(psx)
        exa01 = extract_hop1(0, 1, ps)
        y1t_g1 = {0: y1_stage(0, 0, exa00), 1: y1_stage(0, 1, exa01)}
        x2pass()
        y2t_g1 = {0: hop2(0, 0), 1: hop2(0, 1)}
        rsbs, cts = {}, {}
        ctn = None
        zs = {}
        zs[0] = gate_mm(0, y1t_g1, y2t_g1)
        zs[1] = gate_mm(1, y1t_g1, y2t_g1)
        rsbs[0], cts[0], ctn = gate_post(0, zs[0], ctn)
        zs[2] = gate_mm(2, y1t_g1, y2t_g1)
        rsbs[1], cts[1], ctn = gate_post(1, zs[1], ctn)
        zs[3] = gate_mm(3, y1t_g1, y2t_g1)
        rsbs[2], cts[2], ctn = gate_post(2, zs[2], ctn)
        rsbs[3], cts[3], ctn = gate_post(3, zs[3], ctn)

        # ======================= GCN 2 (update) =======================
        ps = sweep_g2(0, col_outer=True)
        exa10 = extract_hop1(1, 0, ps)
        ps = sweep_g2(1, col_outer=False)
        exa11 = extract_hop1(1, 1, ps)
        y1t_g2 = {0: y1_stage(1, 0, exa10), 1: y1_stage(1, 1, exa11)}
        y2t_g2 = {0: hop2(1, 0), 1: hop2(1, 1)}
        hs = {}
        hs[0] = update_mm(0, cts[0], y1t_g2, y2t_g2)
        hs[1] = update_mm(1, cts[1], y1t_g2, y2t_g2)
        update_post(0, rsbs[0], hs[0])
        hs[2] = update_mm(2, cts[2], y1t_g2, y2t_g2)
        update_post(1, rsbs[1], hs[1])
        hs[3] = update_mm(3, cts[3], y1t_g2, y2t_g2)
        update_post(2, rsbs[2], hs[2])
        update_post(3, rsbs[3], hs[3])


def prepare_in_maps(x, state, support0, support1, W_gate, b_gate,
                    W_update, b_update):
    F8NP = mybir.dt.np(F8)
    BFNP = mybir.dt.np(BF16)
    x_f = np.asarray(x, dtype=np.float32)
    state_f = np.asarray(state, dtype=np.float32)

    st_nm = state_f.transpose(1, 0, 2).reshape(N, SN).astype(F8NP)
    st_dev = np.ascontiguousarray(
        st_nm.reshape(NQ, NT, 128, SN).transpose(0, 2, 1, 3))
    x_nm = x_f.transpose(1, 0, 2).reshape(N, XC).astype(F8NP)
    xm_dev = np.ascontiguousarray(
        x_nm.reshape(NQ, NT, 128, XC).transpose(0, 2, 1, 3))

    # feature-major direct input, rows [state(64); x(2)]
    sxT = np.concatenate([state_f, x_f], axis=-1).transpose(2, 0, 1)

    def fold(W):
        Wf = np.array(W, dtype=np.float32)
        Wf[0:C] -= Wf[2 * C:3 * C] + Wf[4 * C:5 * C]
        Wf[2 * C:3 * C] *= 2.0
        Wf[4 * C:5 * C] *= 2.0
        Wf[C:2 * C] /= SA
        Wf[3 * C:4 * C] /= SA
        Wf[2 * C:3 * C] /= SA * SA
        Wf[4 * C:5 * C] /= SA * SA
        blocks = [Wf[j * C:(j + 1) * C] for j in range(5)]
        W74 = np.concatenate(
            [blocks[0][DIN:], blocks[0][:DIN]]
            + [blk[:DIN] for blk in blocks[1:]], axis=0)
        W64 = np.stack([blk[DIN:] for blk in blocks[1:]], axis=1)
        # doubled rows: the dense stage reads odd batches at partition
        # base 64, so each W64 block is replicated at rows 64:128
        W64d = np.concatenate([W64, W64], axis=0)
        return (np.ascontiguousarray(W74).astype(BFNP),
                np.ascontiguousarray(W64d).astype(BFNP))

    Wg74, Wg64 = fold(W_gate)
    Wu74, Wu64 = fold(W_update)
    bg = np.ascontiguousarray(b_gate, dtype=np.float32).reshape(2 * H, 1)
    bu = np.ascontiguousarray(b_update, dtype=np.float32).reshape(H, 1)

    s_scaled = [
        (np.asarray(support0, dtype=np.float32) * SA).astype(F8NP),
        (np.asarray(support1, dtype=np.float32) * SA).astype(F8NP),
    ]

    in_maps = []
    for r in range(NCORES):
        n0 = r * NOWN
        Ts_dev = np.stack([
            np.ascontiguousarray(sc.T[:, n0:n0 + NOWN])
            .reshape(NQ, NT, 128, NOWN).transpose(0, 2, 1, 3)
            for sc in s_scaled])
        in_maps.append({
            "Ts": np.ascontiguousarray(Ts_dev),
            "st": st_dev,
            "xm": xm_dev,
            "xsT": np.ascontiguousarray(
                sxT[:, :, n0:n0 + NOWN]).astype(BFNP),
            "Wg74": Wg74, "Wg64": Wg64, "Wu74": Wu74, "Wu64": Wu64,
            "bg": bg, "bu": bu,
        })
    return in_maps


def assemble_output(results):
    out = np.empty((B, N, H), dtype=np.float32)
    for r in range(NCORES):
        n0 = r * NOWN
        out[:, n0:n0 + NOWN, :] = results[r]["outT"].transpose(0, 2, 1)
    return out


def get_nc():
    if "nc" not in _NC_CACHE:
        _NC_CACHE["nc"] = build_nc()
    return _NC_CACHE["nc"]


def kernel(x, state, support0, support1, W_gate, b_gate, W_update,
           b_update):
    nc = get_nc()
    in_maps = prepare_in_maps(x, state, support0, support1,
                              W_gate, b_gate, W_update, b_update)
    prev = os.environ.get("BASS_NEVER_TRACE")
    os.environ["BASS_NEVER_TRACE"] = "1"
    try:
        res = run_bass_kernel_spmd(nc, in_maps, list(range(NCORES)),
                                   trace=False)
    finally:
        if prev is None:
            os.environ.pop("BASS_NEVER_TRACE", None)
        else:
            os.environ["BASS_NEVER_TRACE"] = prev
    return assemble_output(res.results)
